# revision 24
# baseline (speedup 1.0000x reference)
"""GRU cell kernel for Trainium2, data-parallel across 8 NeuronCores.

Per core: batch shard of 1024 rows; weights replicated.
  u  = sigmoid(x @ Wxu + h @ Whu + bu)
  r  = sigmoid(x @ Wxr + h @ Whr + br)
  c' = tanh  (x @ Wxc + (h*r) @ Whc + bc)
  c  = u*c' + (1-u)*h

The PE does 768 512-col bf16 matmuls (213 ns streaming floor each at
2.4 GHz -> 164 us/core); everything else hides behind that stream,
which runs at its 216 ns/MM issue floor:
  - consecutive matmuls accumulate into the SAME psum bank (k-inner):
    switching the psum write target every matmul costs ~45 ns of PE
    micro-idle (measured via a k-outer variant), which is why
    LDWEIGHTS-sharing loop orders lose
  - a post-pass strips the per-matmul semaphore increment (a
    serialized ~22 ns EVT_SEM write, measured: 235->216 ns/MM) from
    every matmul nobody waits on, renumbering the surviving wait
    thresholds; every wait target stays an updater so no wait can
    point past its producer
  - host pre-casts to bf16 and lays every DRAM tensor out "p-major"
    ([128, 8192], 16 KB contiguous rows) so the bulk loads ride few
    big-descriptor DMAs (~0.42 MB/us aggregate vs ~0.26 with 2 KB
    descriptors); the r working set streams in k-progressive chunks
    on both HWDGE queues, x/h at the same k so both queues drain in
    lockstep; the first ~30 us are HBM-delivery-bound (8 cores pull
    replicated weights simultaneously), so the r gate runs as a
    k-major wave over all 8 psum banks -- the widest wave absorbs the
    most matmuls into the unavoidable DMA window
  - warm-up matmuls on a first-arriving 32 KB weight micro-chunk
    (K=128 stationary; K=1 does NOT register as PE-busy for the HAM
    clock gate) ramp the clock to 2.4 GHz while the r set lands
  - r gate runs transposed (W stationary) so its bias is per-partition
    and rh^T lands in the layout the c-gate needs as stationary
  - no SBUF slot reuse for weights (all six stay resident): no WAR
    semaphores against the matmul stream
  - u/c gates run per 128-row chunk with rotating uN/qN pools; the
    final c groups taper to 256 wide so the last drain chain is short,
    with the last store split across both DGE queues
  - fp8 was evaluated and rejected: e4m3 on all gates gives 7e-2 rel
    err, e4m3 on just the r weights 2.1e-2, vs the 2e-2 budget (bf16
    sits at 5.9e-3)
"""

import os
import sys

import numpy as np

B = 8192
E = 1024
H = 1024
NCORES = 8
B_SH = B // NCORES  # 1024 rows per core

P = 128
KE = E // P   # 8 contraction chunks per side
NJ = H // P   # 8 output feature chunks
BN = 512      # moving free-dim per matmul / psum tile width
NB = B_SH // BN  # 2
NWARM = 16

W_NAMES = ("Wxu", "Whu", "Wxr", "Whr", "Wxc", "Whc")
B_NAMES = ("bu", "br", "bc")

_NC_CACHE = {}


def _ensure_paths():
    for p in ("/opt/trn_rl_repo", "/root/.axon_site/_ro/trn_rl_repo"):
        if os.path.isdir(p) and p not in sys.path:
            sys.path.insert(0, p)


def _build_nc():
    import concourse.bass as bass
    import concourse.mybir as mybir
    from concourse.tile import TileContext

    f32 = mybir.dt.float32
    f16 = mybir.dt.float16
    bf16 = mybir.dt.bfloat16
    AF = mybir.ActivationFunctionType

    f8 = mybir.dt.float8e4

    nc = bass.Bass()
    # p-major [128, 8192] layouts: column block k holds chunk k (16KB rows)
    xT_d = nc.dram_tensor("inputT", [P, KE * B_SH], bf16, kind="ExternalInput")
    hT_d = nc.dram_tensor("hiddenT", [P, KE * B_SH], bf16, kind="ExternalInput")
    hN_d = nc.dram_tensor("hiddenN", [P, KE * H], bf16, kind="ExternalInput")
    w_d = {n: nc.dram_tensor(n, [P, KE * H], bf16, kind="ExternalInput") for n in W_NAMES}
    b_d = {n: nc.dram_tensor(n, [1, H], f32, kind="ExternalInput") for n in B_NAMES}
    out_d = nc.dram_tensor("output", [B_SH, H], f32, kind="ExternalOutput")

    def csl(k, inner):  # column slice for chunk k of a p-major tile
        return slice(k * B_SH + inner.start, k * B_SH + inner.stop)

    with TileContext(nc) as tc:
        with (
            tc.tile_pool(name="sb", bufs=1) as sb,
            tc.tile_pool(name="psum", bufs=1, space="PSUM") as pp,
        ):
            xT = sb.tile([P, KE * B_SH], bf16, tag="xT", bufs=1)
            hT = sb.tile([P, KE * B_SH], bf16, tag="hT", bufs=1)
            hN = sb.tile([P, KE * H], bf16, tag="hN", bufs=1)
            rhT = sb.tile([P, KE * B_SH], bf16, tag="rhT", bufs=1)
            wsb = {
                n: sb.tile([P, KE * H], bf16, tag=f"w_{n}", bufs=1, name=f"w_{n}")
                for n in W_NAMES
            }

            def psn(name):  # 512-wide psum tile (1 bank)
                return pp.tile([P, BN], f32, tag="mm", bufs=8, name=name)

            # ---- head DMAs: r working set, k-progressive, x/h split across
            # the two HWDGE queues so they drain in lockstep.  A 32KB micro
            # chunk of the r weights lands first to feed the warm-up MMs ----
            nc.sync.dma_start(wsb["Wxr"][:, 0:P], w_d["Wxr"][:, 0:P])
            nc.scalar.dma_start(wsb["Whr"][:, 0:P], w_d["Whr"][:, 0:P])
            nc.sync.dma_start(xT[:, 0:1024], xT_d[:, 0:1024])
            nc.scalar.dma_start(hT[:, 0:1024], hT_d[:, 0:1024])
            nc.sync.dma_start(wsb["Wxr"][:, P:1024], w_d["Wxr"][:, P:1024])
            nc.scalar.dma_start(wsb["Whr"][:, P:1024], w_d["Whr"][:, P:1024])
            CH = ((1024, 2048), (2048, 4096), (4096, 6144), (6144, 8192))
            for lo, hi in CH:
                nc.sync.dma_start(xT[:, lo:hi], xT_d[:, lo:hi])
                nc.sync.dma_start(wsb["Wxr"][:, lo:hi], w_d["Wxr"][:, lo:hi])
                nc.scalar.dma_start(hT[:, lo:hi], hT_d[:, lo:hi])
                nc.scalar.dma_start(wsb["Whr"][:, lo:hi], w_d["Whr"][:, lo:hi])

            # small bias/constant loads (needed from ~35us on)
            ones = sb.tile([1, BN], bf16, tag="ones", bufs=1)
            nc.vector.memset(ones[:], 1.0)
            br_t = sb.tile([P, NJ], f32, tag="br_t", bufs=1)
            nc.scalar.dma_start(
                br_t[:], b_d["br"][0:1, :].rearrange("a (j p) -> p (a j)", p=P)
            )
            brow_f = {}
            for nm in ("bu", "bc"):
                rf = sb.tile([1, H], f32, tag="brow_f", bufs=2, name=f"rf_{nm}")
                nc.scalar.dma_start(rf[:], b_d[nm][0:1, :])
                brow_f[nm] = rf

            # remaining weights / natural h, in need-by order
            nc.sync.dma_start(wsb["Wxu"][:], w_d["Wxu"][:])
            nc.sync.dma_start(wsb["Wxc"][:], w_d["Wxc"][:])
            nc.scalar.dma_start(wsb["Whu"][:], w_d["Whu"][:])
            nc.scalar.dma_start(hN[:], hN_d[:])
            nc.scalar.dma_start(wsb["Whc"][:], w_d["Whc"][:])

            # ---- warm-up: K=128, N=128 matmuls on the first-arriving weight
            # micro chunk ramp HAM toward full clock; dummy activations
            # preload the Sigmoid/Tanh tables ----
            warm = psn("warm")
            warm_o = sb.tile([1, 8], f32, tag="warm_o", bufs=2)
            nc.scalar.activation(warm_o[:], ones[0:1, 0:8], AF.Sigmoid)
            nc.scalar.activation(warm_o[:], ones[0:1, 0:8], AF.Tanh)
            for i in range(NWARM):
                nc.tensor.matmul(
                    warm[:, 0:P], wsb["Wxr"][:, 0:P], wsb["Wxr"][:, 0:P],
                    start=True, stop=True,
                )

            # bias rows to bf16 early (Vector is idle here)
            brow_b = {}
            for nm in ("bu", "bc"):
                rb = sb.tile([1, H], bf16, tag="brow_b", bufs=2, name=f"rb_{nm}")
                nc.vector.tensor_copy(rb[:], brow_f[nm][:])
                brow_b[nm] = rb

            # ---- r gate (transposed out): r^T*h^T into rhT ----
            def r_sigmoid(j, n, ps):
                nsl = slice(n * BN, (n + 1) * BN)
                osl = csl(j, nsl)
                nc.scalar.activation(
                    rhT[:, osl], ps[:], AF.Sigmoid, bias=br_t[:, j : j + 1]
                )
                nc.vector.tensor_mul(rhT[:, osl], rhT[:, osl], hT[:, osl])

            # first half: k-major over an 8-tile wave (16 MMs per k-level,
            # x and h at the same k so both queues feed in lockstep; the
            # wave is delivery-bound, so the wider it is the more matmuls
            # hide inside the DMA window)
            wave = [(j, n) for j in (0, 1, 2, 3) for n in range(NB)]
            wtiles = {jn: psn(f"ps_r{jn[0]}{jn[1]}") for jn in wave}
            for k in range(KE):
                for j, n in wave:
                    jsl = slice(j * P, (j + 1) * P)
                    nsl = slice(n * BN, (n + 1) * BN)
                    ps = wtiles[(j, n)]
                    nc.tensor.matmul(
                        ps[:], wsb["Wxr"][:, csl(k, jsl)], xT[:, csl(k, nsl)],
                        start=(k == 0), stop=False,
                    )
                    nc.tensor.matmul(
                        ps[:], wsb["Whr"][:, csl(k, jsl)], hT[:, csl(k, nsl)],
                        start=False, stop=(k == KE - 1),
                    )
            for jn in wave:
                r_sigmoid(jn[0], jn[1], wtiles[jn])

            # second half: weights resident; 16 consecutive MMs per psum tile
            def r_serial(j, ns=(0, 1)):
                jsl = slice(j * P, (j + 1) * P)
                for n in ns:
                    nsl = slice(n * BN, (n + 1) * BN)
                    ps = psn(f"ps_r{j}{n}")
                    for k in range(KE):
                        nc.tensor.matmul(
                            ps[:], wsb["Wxr"][:, csl(k, jsl)], xT[:, csl(k, nsl)],
                            start=(k == 0), stop=False,
                        )
                    for k in range(KE):
                        nc.tensor.matmul(
                            ps[:], wsb["Whr"][:, csl(k, jsl)], hT[:, csl(k, nsl)],
                            start=False, stop=(k == KE - 1),
                        )
                    r_sigmoid(j, n, ps)

            r_serial(4)
            r_serial(5)

            # broadcast bias rows into [P, H] tiles via K=1 matmuls (PE is
            # warm; placed here so psum pool pressure stays <= 8 banks)
            bcast = {}
            for nm in ("bu", "bc"):
                rb = brow_b[nm]
                bt = sb.tile([P, H], bf16, tag=f"bcast_{nm}", name=f"bcast_{nm}", bufs=1)
                for n in range(NB):
                    nsl = slice(n * BN, (n + 1) * BN)
                    ps = psn(f"psb_{nm}{n}")
                    nc.tensor.matmul(ps[:], ones[0:1, 0:P], rb[0:1, nsl], start=True, stop=True)
                    nc.vector.tensor_copy(bt[:, nsl], ps[:])
                bcast[nm] = bt

            for j in range(6, NJ):
                r_serial(j)

            # ---- u gate then c gate per 128-row b-chunk (k-inner; uN/qN
            # rotate through small pools, consumed by the same chunk's blend)
            for b in range(B_SH // P):
                bsl = slice(b * P, (b + 1) * P)
                uN = sb.tile([P, H], f16, tag="uN", bufs=2, name=f"uN{b}")
                qN = sb.tile([P, H], f16, tag="qN", bufs=2, name=f"qN{b}")
                for n in range(NB):
                    nsl = slice(n * BN, (n + 1) * BN)
                    ps = psn(f"ps_u{b}{n}")
                    for k in range(KE):
                        nc.tensor.matmul(
                            ps[:], xT[:, csl(k, bsl)], wsb["Wxu"][:, csl(k, nsl)],
                            start=(k == 0), stop=False,
                        )
                    for k in range(KE):
                        nc.tensor.matmul(
                            ps[:], hT[:, csl(k, bsl)], wsb["Whu"][:, csl(k, nsl)],
                            start=False, stop=(k == KE - 1),
                        )
                    nc.vector.tensor_add(ps[:], ps[:], bcast["bu"][:, nsl])
                    nc.scalar.activation(uN[:, nsl], ps[:], AF.Sigmoid)
                # q = h - u*h  (so the blend is c = u*c' + q)
                hsl = csl(b, slice(0, H))
                nc.vector.tensor_mul(qN[:], uN[:], hN[:, hsl])
                nc.vector.tensor_sub(qN[:], hN[:, hsl], qN[:])
                # c gate + blend + store; the last b-chunk tapers to
                # 256-wide psum groups so the final drain chain is short
                last_b = b == B_SH // P - 1
                groups = ((0, 512), (512, 768), (768, 1024)) if last_b else ((0, 512), (512, 1024))
                for gi, (g0, g1) in enumerate(groups):
                    gw = g1 - g0
                    nsl = slice(g0, g1)
                    ps = psn(f"ps_c{b}{gi}")
                    for k in range(KE):
                        nc.tensor.matmul(
                            ps[:, :gw], xT[:, csl(k, bsl)], wsb["Wxc"][:, csl(k, nsl)],
                            start=(k == 0), stop=False,
                        )
                    for k in range(KE):
                        nc.tensor.matmul(
                            ps[:, :gw], rhT[:, csl(k, bsl)], wsb["Whc"][:, csl(k, nsl)],
                            start=False, stop=(k == KE - 1),
                        )
                    nc.vector.tensor_add(ps[:, :gw], ps[:, :gw], bcast["bc"][:, nsl])
                    cc = sb.tile([P, BN], f32, tag="cc", bufs=2)
                    nc.scalar.activation(cc[:, :gw], ps[:, :gw], AF.Tanh)
                    nc.vector.tensor_mul(cc[:, :gw], cc[:, :gw], uN[:, nsl])
                    nc.vector.tensor_add(cc[:, :gw], cc[:, :gw], qN[:, nsl])
                    if last_b and gi == len(groups) - 1:
                        # split the last store across both DGE queues
                        nc.sync.dma_start(out_d[b * P : b * P + P // 2, nsl], cc[0 : P // 2, :gw])
                        nc.scalar.dma_start(out_d[b * P + P // 2 : (b + 1) * P, nsl], cc[P // 2 :, :gw])
                    else:
                        nc.sync.dma_start(out_d[bsl, nsl], cc[:, :gw])

    _split_matmul_waits(nc, mybir)
    if not os.environ.get("KERNEL_NO_DEDUP"):
        _dedup_ldweights(nc, mybir)
    if not os.environ.get("KERNEL_NO_STRIP"):
        _strip_mm_updates(nc, mybir)
    return nc


def _split_matmul_waits(nc, mybir):
    """Walrus codegen allows only one sync-wait on a Matmult (it lowers to an
    LDW+MM pair).  Spill extra waits onto a PE NoOp placed just before."""
    n_fixed = 0
    blocks = list(nc.m.functions[0].blocks)
    origs = [list(b.instructions) for b in blocks]
    spill_nops = {}  # id(inst) -> [nop insts]
    for orig in origs:
        for inst in orig:
            si = inst.sync_info
            if (
                si is not None
                and si.on_wait
                and len(si.on_wait) > 1
            ):
                waits = list(si.on_wait)
                eng = nc.engines[inst.engine]
                nops = []
                for w in waits[:-1]:
                    nop = eng.nop(hint="waitspill").ins
                    nop.sync_info = mybir.SyncInfo(on_wait=[w], on_update=[])
                    nops.append(nop)
                inst.sync_info = mybir.SyncInfo(
                    on_wait=waits[-1:], on_update=list(si.on_update or [])
                )
                spill_nops[id(inst)] = nops
                n_fixed += 1
    for blk, orig in zip(blocks, origs):
        new_list = []
        for inst in orig:
            if id(inst) in spill_nops:
                new_list.extend(spill_nops[id(inst)])
            new_list.append(inst)
        # rebuilding from `orig` also drops any freshly created nops that
        # bass appended to this block's tail
        blk.instructions[:] = new_list
    return n_fixed


def _dedup_ldweights(nc, mybir):
    """Delete InstLdweights that reload the stationary operand already in
    the PE array (same AP as the previous Ldweights).  The duplicate's
    waits move onto the next PE instruction."""
    n_removed = 0
    for blk in nc.m.functions[0].blocks:
        insts = list(blk.instructions)
        keep = []
        last_sig = None
        pending_waits = []
        for inst in insts:
            tn = type(inst).__name__
            eng = str(inst.engine)
            if eng != "EngineType.PE":
                keep.append(inst)
                continue
            if tn == "InstLdweights":
                sig = (
                    repr(inst.ins[0]),
                    getattr(inst, "is_transpose", None),
                    getattr(inst, "perf_mode", None),
                    getattr(inst, "tile_position", None),
                )
                if sig == last_sig:
                    si = inst.sync_info
                    assert not (si and si.on_update), "dup LDW carries update"
                    if si and si.on_wait:
                        pending_waits.extend(si.on_wait)
                    n_removed += 1
                    continue
                last_sig = sig
                keep.append(inst)
            else:
                if tn not in ("InstMatmult", "InstNoOp", "InstRegisterMove"):
                    last_sig = None
                if pending_waits:
                    si = inst.sync_info
                    w = list(si.on_wait) if si and si.on_wait else []
                    seen = {(x.id, x.wait_value) for x in w}
                    for x in pending_waits:
                        if (x.id, x.wait_value) not in seen:
                            w.append(x)
                            seen.add((x.id, x.wait_value))
                    inst.sync_info = mybir.SyncInfo(
                        on_wait=w,
                        on_update=list(si.on_update or []) if si else [],
                    )
                    pending_waits = []
                keep.append(inst)
        blk.instructions[:] = keep
    # spilled multi-waits may have appeared on Matmults again
    _split_matmul_waits(nc, mybir)
    return n_removed


def _strip_mm_updates(nc, mybir):
    """Every PE Matmult increments the PE engine semaphore (a serialized
    ~22ns EVT_SEM write).  Keep the increment only on matmuls some wait
    actually needs, renumbering all wait thresholds on that semaphore."""
    import bisect

    blocks = list(nc.m.functions[0].blocks)
    sem_id = None
    for blk in blocks:
        for inst in blk.instructions:
            if type(inst).__name__ == "InstMatmult":
                si = inst.sync_info
                if si and si.on_update:
                    assert len(si.on_update) == 1
                    u = si.on_update[0]
                    assert u.update_mode == "sem-inc" and u.update_value == 1
                    if sem_id is None:
                        sem_id = u.id
                    else:
                        assert u.id == sem_id
    if sem_id is None:
        return 0
    updaters = []
    for blk in blocks:
        for inst in blk.instructions:
            si = inst.sync_info
            if si and si.on_update:
                for u in si.on_update:
                    if u.id == sem_id:
                        assert u.update_mode == "sem-inc" and u.update_value == 1
                        updaters.append(inst)
    n_upd = len(updaters)
    waits = []
    for blk in blocks:
        for inst in blk.instructions:
            si = inst.sync_info
            if si and si.on_wait:
                for w in si.on_wait:
                    if w.id == sem_id:
                        assert w.wait_mode == "sem-ge-imm", w.wait_mode
                        assert 1 <= w.wait_value <= n_upd
                        waits.append(w)
    keep_idx = sorted({w.wait_value - 1 for w in waits} | {n_upd - 1})
    for w in waits:
        w.wait_value = bisect.bisect_left(keep_idx, w.wait_value - 1) + 1
    keep_set = set(keep_idx)
    n_stripped = 0
    for i, inst in enumerate(updaters):
        if i in keep_set:
            continue
        si = inst.sync_info
        new_upd = [u for u in si.on_update if u.id != sem_id]
        inst.sync_info = mybir.SyncInfo(
            on_wait=list(si.on_wait or []), on_update=new_upd
        )
        n_stripped += 1
    return n_stripped


def get_nc():
    if "nc" not in _NC_CACHE:
        _ensure_paths()
        _NC_CACHE["nc"] = _build_nc()
    return _NC_CACHE["nc"]


def _pmajor(a):
    # [1024, N] -> [128, 8*N] where out[p, k*N+j] = a[k*128+p, j]
    return np.ascontiguousarray(
        a.reshape(KE, P, a.shape[1]).transpose(1, 0, 2).reshape(P, KE * a.shape[1])
    )


def make_in_maps(inputs):
    import ml_dtypes

    bf16 = ml_dtypes.bfloat16
    x = np.asarray(inputs["input"], dtype=np.float32).astype(bf16)
    h = np.asarray(inputs["hidden_state"], dtype=np.float32).astype(bf16)
    xT = x.T  # [E, B]
    hT = h.T
    shared = {
        n: _pmajor(np.asarray(inputs[n], dtype=np.float32).astype(bf16))
        for n in W_NAMES
    }
    shared.update(
        {n: np.ascontiguousarray(np.asarray(inputs[n], dtype=np.float32)) for n in B_NAMES}
    )
    in_maps = []
    for c in range(NCORES):
        sl = slice(c * B_SH, (c + 1) * B_SH)
        m = {
            "inputT": _pmajor(np.ascontiguousarray(xT[:, sl])),
            "hiddenT": _pmajor(np.ascontiguousarray(hT[:, sl])),
            "hiddenN": _pmajor(np.ascontiguousarray(h[sl])),
        }
        m.update(shared)
        in_maps.append(m)
    return in_maps


def kernel(**inputs):
    _ensure_paths()
    from concourse.bass_utils import run_bass_kernel_spmd

    nc = get_nc()
    res = run_bass_kernel_spmd(nc, make_in_maps(inputs), list(range(NCORES)))
    out = np.concatenate(
        [np.asarray(res.results[c]["output"]) for c in range(NCORES)], axis=0
    )
    return out.astype(np.float32)


# revision 25
# speedup vs baseline: 1.0204x; 1.0204x over previous
"""GRU cell kernel for Trainium2, data-parallel across 8 NeuronCores.

Per core: batch shard of 1024 rows; weights replicated.
  u  = sigmoid(x @ Wxu + h @ Whu + bu)
  r  = sigmoid(x @ Wxr + h @ Whr + br)
  c' = tanh  (x @ Wxc + (h*r) @ Whc + bc)
  c  = u*c' + (1-u)*h

The PE does 768 512-col bf16 matmuls (213 ns streaming floor each at
2.4 GHz -> 164 us/core); everything else hides behind that stream,
which runs at its 216 ns/MM issue floor:
  - consecutive matmuls accumulate into the SAME psum bank (k-inner):
    switching the psum write target every matmul costs ~45 ns of PE
    micro-idle (measured via a k-outer variant), which is why
    LDWEIGHTS-sharing loop orders lose
  - a post-pass strips the per-matmul semaphore increment (a
    serialized ~22 ns EVT_SEM write, measured: 235->216 ns/MM) from
    every matmul nobody waits on, renumbering the surviving wait
    thresholds; every wait target stays an updater so no wait can
    point past its producer
  - host pre-casts to bf16 and lays every DRAM tensor out "p-major"
    ([128, 8192], 16 KB contiguous rows) so the bulk loads ride few
    big-descriptor DMAs (~0.42 MB/us aggregate vs ~0.26 with 2 KB
    descriptors); the r working set streams in k-progressive chunks
    on both HWDGE queues, x/h at the same k so both queues drain in
    lockstep; the first ~30 us are HBM-delivery-bound (8 cores pull
    replicated weights simultaneously), so the r gate runs as a
    k-major wave over 7 psum banks -- a wide wave absorbs the most
    matmuls into the unavoidable DMA window, and the 8th bank keeps
    the wave-drain handoff off the critical path
  - warm-up matmuls on a first-arriving 32 KB weight micro-chunk
    (K=128 stationary; K=1 does NOT register as PE-busy for the HAM
    clock gate) ramp the clock to 2.4 GHz while the r set lands
  - r gate runs transposed (W stationary) so its bias is per-partition
    and rh^T lands in the layout the c-gate needs as stationary
  - no SBUF slot reuse for weights (all six stay resident): no WAR
    semaphores against the matmul stream
  - u/c gates run per 128-row chunk with rotating uN/qN pools; the
    final c groups taper to 256 wide so the last drain chain is short,
    with the last store split across both DGE queues
  - fp8 was evaluated and rejected: e4m3 on all gates gives 7e-2 rel
    err, e4m3 on just the r weights 2.1e-2, vs the 2e-2 budget (bf16
    sits at 5.9e-3)
"""

import os
import sys

import numpy as np

B = 8192
E = 1024
H = 1024
NCORES = 8
B_SH = B // NCORES  # 1024 rows per core

P = 128
KE = E // P   # 8 contraction chunks per side
NJ = H // P   # 8 output feature chunks
BN = 512      # moving free-dim per matmul / psum tile width
NB = B_SH // BN  # 2
NWARM = 16

W_NAMES = ("Wxu", "Whu", "Wxr", "Whr", "Wxc", "Whc")
B_NAMES = ("bu", "br", "bc")

_NC_CACHE = {}


def _ensure_paths():
    for p in ("/opt/trn_rl_repo", "/root/.axon_site/_ro/trn_rl_repo"):
        if os.path.isdir(p) and p not in sys.path:
            sys.path.insert(0, p)


def _build_nc():
    import concourse.bass as bass
    import concourse.mybir as mybir
    from concourse.tile import TileContext

    f32 = mybir.dt.float32
    f16 = mybir.dt.float16
    bf16 = mybir.dt.bfloat16
    AF = mybir.ActivationFunctionType

    f8 = mybir.dt.float8e4

    nc = bass.Bass()
    # p-major [128, 8192] layouts: column block k holds chunk k (16KB rows)
    xT_d = nc.dram_tensor("inputT", [P, KE * B_SH], bf16, kind="ExternalInput")
    hT_d = nc.dram_tensor("hiddenT", [P, KE * B_SH], bf16, kind="ExternalInput")
    hN_d = nc.dram_tensor("hiddenN", [P, KE * H], bf16, kind="ExternalInput")
    w_d = {n: nc.dram_tensor(n, [P, KE * H], bf16, kind="ExternalInput") for n in W_NAMES}
    b_d = {n: nc.dram_tensor(n, [1, H], f32, kind="ExternalInput") for n in B_NAMES}
    out_d = nc.dram_tensor("output", [B_SH, H], f32, kind="ExternalOutput")

    def csl(k, inner):  # column slice for chunk k of a p-major tile
        return slice(k * B_SH + inner.start, k * B_SH + inner.stop)

    with TileContext(nc) as tc:
        with (
            tc.tile_pool(name="sb", bufs=1) as sb,
            tc.tile_pool(name="psum", bufs=1, space="PSUM") as pp,
        ):
            xT = sb.tile([P, KE * B_SH], bf16, tag="xT", bufs=1)
            hT = sb.tile([P, KE * B_SH], bf16, tag="hT", bufs=1)
            hN = sb.tile([P, KE * H], bf16, tag="hN", bufs=1)
            rhT = sb.tile([P, KE * B_SH], bf16, tag="rhT", bufs=1)
            wsb = {
                n: sb.tile([P, KE * H], bf16, tag=f"w_{n}", bufs=1, name=f"w_{n}")
                for n in W_NAMES
            }

            def psn(name):  # 512-wide psum tile (1 bank)
                return pp.tile([P, BN], f32, tag="mm", bufs=8, name=name)

            # ---- head DMAs: r working set, k-progressive, x/h split across
            # the two HWDGE queues so they drain in lockstep.  A 32KB micro
            # chunk of the r weights lands first to feed the warm-up MMs ----
            nc.sync.dma_start(wsb["Wxr"][:, 0:P], w_d["Wxr"][:, 0:P])
            nc.scalar.dma_start(wsb["Whr"][:, 0:P], w_d["Whr"][:, 0:P])
            nc.sync.dma_start(xT[:, 0:1024], xT_d[:, 0:1024])
            nc.scalar.dma_start(hT[:, 0:1024], hT_d[:, 0:1024])
            nc.sync.dma_start(wsb["Wxr"][:, P:1024], w_d["Wxr"][:, P:1024])
            nc.scalar.dma_start(wsb["Whr"][:, P:1024], w_d["Whr"][:, P:1024])
            CH = ((1024, 2048), (2048, 4096), (4096, 6144), (6144, 8192))
            for lo, hi in CH:
                nc.sync.dma_start(xT[:, lo:hi], xT_d[:, lo:hi])
                nc.sync.dma_start(wsb["Wxr"][:, lo:hi], w_d["Wxr"][:, lo:hi])
                nc.scalar.dma_start(hT[:, lo:hi], hT_d[:, lo:hi])
                nc.scalar.dma_start(wsb["Whr"][:, lo:hi], w_d["Whr"][:, lo:hi])

            # small bias/constant loads (needed from ~35us on)
            ones = sb.tile([1, BN], bf16, tag="ones", bufs=1)
            nc.vector.memset(ones[:], 1.0)
            br_t = sb.tile([P, NJ], f32, tag="br_t", bufs=1)
            nc.scalar.dma_start(
                br_t[:], b_d["br"][0:1, :].rearrange("a (j p) -> p (a j)", p=P)
            )
            brow_f = {}
            for nm in ("bu", "bc"):
                rf = sb.tile([1, H], f32, tag="brow_f", bufs=2, name=f"rf_{nm}")
                nc.scalar.dma_start(rf[:], b_d[nm][0:1, :])
                brow_f[nm] = rf

            # remaining weights / natural h, in need-by order
            nc.sync.dma_start(wsb["Wxu"][:], w_d["Wxu"][:])
            nc.sync.dma_start(wsb["Wxc"][:], w_d["Wxc"][:])
            nc.scalar.dma_start(wsb["Whu"][:], w_d["Whu"][:])
            nc.scalar.dma_start(hN[:], hN_d[:])
            nc.scalar.dma_start(wsb["Whc"][:], w_d["Whc"][:])

            # ---- warm-up: K=128, N=128 matmuls on the first-arriving weight
            # micro chunk ramp HAM toward full clock; dummy activations
            # preload the Sigmoid/Tanh tables ----
            warm = psn("warm")
            warm_o = sb.tile([1, 8], f32, tag="warm_o", bufs=2)
            nc.scalar.activation(warm_o[:], ones[0:1, 0:8], AF.Sigmoid)
            nc.scalar.activation(warm_o[:], ones[0:1, 0:8], AF.Tanh)
            for i in range(NWARM):
                nc.tensor.matmul(
                    warm[:, 0:P], wsb["Wxr"][:, 0:P], wsb["Wxr"][:, 0:P],
                    start=True, stop=True,
                )

            # bias rows to bf16 early (Vector is idle here)
            brow_b = {}
            for nm in ("bu", "bc"):
                rb = sb.tile([1, H], bf16, tag="brow_b", bufs=2, name=f"rb_{nm}")
                nc.vector.tensor_copy(rb[:], brow_f[nm][:])
                brow_b[nm] = rb

            # ---- r gate (transposed out): r^T*h^T into rhT ----
            def r_sigmoid(j, n, ps):
                nsl = slice(n * BN, (n + 1) * BN)
                osl = csl(j, nsl)
                nc.scalar.activation(
                    rhT[:, osl], ps[:], AF.Sigmoid, bias=br_t[:, j : j + 1]
                )
                nc.vector.tensor_mul(rhT[:, osl], rhT[:, osl], hT[:, osl])

            # first half: k-major over a 7-tile wave (14 MMs per k-level,
            # x and h at the same k so both queues feed in lockstep).  7
            # tiles + warm = 8 psum banks: the first serial tile reuses the
            # long-free warm bank, giving the first wave drain a full
            # serial-tile of slack before its bank is needed (an 8-tile
            # wave makes that reuse race the ~2us cross-engine semaphore
            # wake latency and intermittently trips a HAM re-throttle)
            wave = [(j, n) for j in (0, 1, 2, 3) for n in range(NB)][:-1]
            wtiles = {jn: psn(f"ps_r{jn[0]}{jn[1]}") for jn in wave}
            for k in range(KE):
                for j, n in wave:
                    jsl = slice(j * P, (j + 1) * P)
                    nsl = slice(n * BN, (n + 1) * BN)
                    ps = wtiles[(j, n)]
                    nc.tensor.matmul(
                        ps[:], wsb["Wxr"][:, csl(k, jsl)], xT[:, csl(k, nsl)],
                        start=(k == 0), stop=False,
                    )
                    nc.tensor.matmul(
                        ps[:], wsb["Whr"][:, csl(k, jsl)], hT[:, csl(k, nsl)],
                        start=False, stop=(k == KE - 1),
                    )
            for jn in wave:
                r_sigmoid(jn[0], jn[1], wtiles[jn])

            # second half: weights resident; 16 consecutive MMs per psum tile
            def r_serial(j, ns=(0, 1)):
                jsl = slice(j * P, (j + 1) * P)
                for n in ns:
                    nsl = slice(n * BN, (n + 1) * BN)
                    ps = psn(f"ps_r{j}{n}")
                    for k in range(KE):
                        nc.tensor.matmul(
                            ps[:], wsb["Wxr"][:, csl(k, jsl)], xT[:, csl(k, nsl)],
                            start=(k == 0), stop=False,
                        )
                    for k in range(KE):
                        nc.tensor.matmul(
                            ps[:], wsb["Whr"][:, csl(k, jsl)], hT[:, csl(k, nsl)],
                            start=False, stop=(k == KE - 1),
                        )
                    r_sigmoid(j, n, ps)

            r_serial(3, ns=(1,))
            r_serial(4)
            r_serial(5)

            # broadcast bias rows into [P, H] tiles via K=1 matmuls (PE is
            # warm; placed here so psum pool pressure stays <= 8 banks)
            bcast = {}
            for nm in ("bu", "bc"):
                rb = brow_b[nm]
                bt = sb.tile([P, H], bf16, tag=f"bcast_{nm}", name=f"bcast_{nm}", bufs=1)
                for n in range(NB):
                    nsl = slice(n * BN, (n + 1) * BN)
                    ps = psn(f"psb_{nm}{n}")
                    nc.tensor.matmul(ps[:], ones[0:1, 0:P], rb[0:1, nsl], start=True, stop=True)
                    nc.vector.tensor_copy(bt[:, nsl], ps[:])
                bcast[nm] = bt

            for j in range(6, NJ):
                r_serial(j)

            # ---- u gate then c gate per 128-row b-chunk (k-inner; uN/qN
            # rotate through small pools, consumed by the same chunk's blend)
            for b in range(B_SH // P):
                bsl = slice(b * P, (b + 1) * P)
                uN = sb.tile([P, H], f16, tag="uN", bufs=2, name=f"uN{b}")
                qN = sb.tile([P, H], f16, tag="qN", bufs=2, name=f"qN{b}")
                for n in range(NB):
                    nsl = slice(n * BN, (n + 1) * BN)
                    ps = psn(f"ps_u{b}{n}")
                    for k in range(KE):
                        nc.tensor.matmul(
                            ps[:], xT[:, csl(k, bsl)], wsb["Wxu"][:, csl(k, nsl)],
                            start=(k == 0), stop=False,
                        )
                    for k in range(KE):
                        nc.tensor.matmul(
                            ps[:], hT[:, csl(k, bsl)], wsb["Whu"][:, csl(k, nsl)],
                            start=False, stop=(k == KE - 1),
                        )
                    nc.vector.tensor_add(ps[:], ps[:], bcast["bu"][:, nsl])
                    nc.scalar.activation(uN[:, nsl], ps[:], AF.Sigmoid)
                # q = h - u*h  (so the blend is c = u*c' + q)
                hsl = csl(b, slice(0, H))
                nc.vector.tensor_mul(qN[:], uN[:], hN[:, hsl])
                nc.vector.tensor_sub(qN[:], hN[:, hsl], qN[:])
                # c gate + blend + store; the last b-chunk tapers to
                # 256-wide psum groups so the final drain chain is short
                last_b = b == B_SH // P - 1
                groups = ((0, 512), (512, 768), (768, 1024)) if last_b else ((0, 512), (512, 1024))
                for gi, (g0, g1) in enumerate(groups):
                    gw = g1 - g0
                    nsl = slice(g0, g1)
                    ps = psn(f"ps_c{b}{gi}")
                    for k in range(KE):
                        nc.tensor.matmul(
                            ps[:, :gw], xT[:, csl(k, bsl)], wsb["Wxc"][:, csl(k, nsl)],
                            start=(k == 0), stop=False,
                        )
                    for k in range(KE):
                        nc.tensor.matmul(
                            ps[:, :gw], rhT[:, csl(k, bsl)], wsb["Whc"][:, csl(k, nsl)],
                            start=False, stop=(k == KE - 1),
                        )
                    nc.vector.tensor_add(ps[:, :gw], ps[:, :gw], bcast["bc"][:, nsl])
                    cc = sb.tile([P, BN], f32, tag="cc", bufs=2)
                    nc.scalar.activation(cc[:, :gw], ps[:, :gw], AF.Tanh)
                    nc.vector.tensor_mul(cc[:, :gw], cc[:, :gw], uN[:, nsl])
                    nc.vector.tensor_add(cc[:, :gw], cc[:, :gw], qN[:, nsl])
                    if last_b and gi == len(groups) - 1:
                        # split the last store across both DGE queues
                        nc.sync.dma_start(out_d[b * P : b * P + P // 2, nsl], cc[0 : P // 2, :gw])
                        nc.scalar.dma_start(out_d[b * P + P // 2 : (b + 1) * P, nsl], cc[P // 2 :, :gw])
                    else:
                        nc.sync.dma_start(out_d[bsl, nsl], cc[:, :gw])

    _split_matmul_waits(nc, mybir)
    if not os.environ.get("KERNEL_NO_DEDUP"):
        _dedup_ldweights(nc, mybir)
    if not os.environ.get("KERNEL_NO_STRIP"):
        _strip_mm_updates(nc, mybir)
    return nc


def _split_matmul_waits(nc, mybir):
    """Walrus codegen allows only one sync-wait on a Matmult (it lowers to an
    LDW+MM pair).  Spill extra waits onto a PE NoOp placed just before."""
    n_fixed = 0
    blocks = list(nc.m.functions[0].blocks)
    origs = [list(b.instructions) for b in blocks]
    spill_nops = {}  # id(inst) -> [nop insts]
    for orig in origs:
        for inst in orig:
            si = inst.sync_info
            if (
                si is not None
                and si.on_wait
                and len(si.on_wait) > 1
            ):
                waits = list(si.on_wait)
                eng = nc.engines[inst.engine]
                nops = []
                for w in waits[:-1]:
                    nop = eng.nop(hint="waitspill").ins
                    nop.sync_info = mybir.SyncInfo(on_wait=[w], on_update=[])
                    nops.append(nop)
                inst.sync_info = mybir.SyncInfo(
                    on_wait=waits[-1:], on_update=list(si.on_update or [])
                )
                spill_nops[id(inst)] = nops
                n_fixed += 1
    for blk, orig in zip(blocks, origs):
        new_list = []
        for inst in orig:
            if id(inst) in spill_nops:
                new_list.extend(spill_nops[id(inst)])
            new_list.append(inst)
        # rebuilding from `orig` also drops any freshly created nops that
        # bass appended to this block's tail
        blk.instructions[:] = new_list
    return n_fixed


def _dedup_ldweights(nc, mybir):
    """Delete InstLdweights that reload the stationary operand already in
    the PE array (same AP as the previous Ldweights).  The duplicate's
    waits move onto the next PE instruction."""
    n_removed = 0
    for blk in nc.m.functions[0].blocks:
        insts = list(blk.instructions)
        keep = []
        last_sig = None
        pending_waits = []
        for inst in insts:
            tn = type(inst).__name__
            eng = str(inst.engine)
            if eng != "EngineType.PE":
                keep.append(inst)
                continue
            if tn == "InstLdweights":
                sig = (
                    repr(inst.ins[0]),
                    getattr(inst, "is_transpose", None),
                    getattr(inst, "perf_mode", None),
                    getattr(inst, "tile_position", None),
                )
                if sig == last_sig:
                    si = inst.sync_info
                    assert not (si and si.on_update), "dup LDW carries update"
                    if si and si.on_wait:
                        pending_waits.extend(si.on_wait)
                    n_removed += 1
                    continue
                last_sig = sig
                keep.append(inst)
            else:
                if tn not in ("InstMatmult", "InstNoOp", "InstRegisterMove"):
                    last_sig = None
                if pending_waits:
                    si = inst.sync_info
                    w = list(si.on_wait) if si and si.on_wait else []
                    seen = {(x.id, x.wait_value) for x in w}
                    for x in pending_waits:
                        if (x.id, x.wait_value) not in seen:
                            w.append(x)
                            seen.add((x.id, x.wait_value))
                    inst.sync_info = mybir.SyncInfo(
                        on_wait=w,
                        on_update=list(si.on_update or []) if si else [],
                    )
                    pending_waits = []
                keep.append(inst)
        blk.instructions[:] = keep
    # spilled multi-waits may have appeared on Matmults again
    _split_matmul_waits(nc, mybir)
    return n_removed


def _strip_mm_updates(nc, mybir):
    """Every PE Matmult increments the PE engine semaphore (a serialized
    ~22ns EVT_SEM write).  Keep the increment only on matmuls some wait
    actually needs, renumbering all wait thresholds on that semaphore."""
    import bisect

    blocks = list(nc.m.functions[0].blocks)
    sem_id = None
    for blk in blocks:
        for inst in blk.instructions:
            if type(inst).__name__ == "InstMatmult":
                si = inst.sync_info
                if si and si.on_update:
                    assert len(si.on_update) == 1
                    u = si.on_update[0]
                    assert u.update_mode == "sem-inc" and u.update_value == 1
                    if sem_id is None:
                        sem_id = u.id
                    else:
                        assert u.id == sem_id
    if sem_id is None:
        return 0
    updaters = []
    for blk in blocks:
        for inst in blk.instructions:
            si = inst.sync_info
            if si and si.on_update:
                for u in si.on_update:
                    if u.id == sem_id:
                        assert u.update_mode == "sem-inc" and u.update_value == 1
                        updaters.append(inst)
    n_upd = len(updaters)
    waits = []
    for blk in blocks:
        for inst in blk.instructions:
            si = inst.sync_info
            if si and si.on_wait:
                for w in si.on_wait:
                    if w.id == sem_id:
                        assert w.wait_mode == "sem-ge-imm", w.wait_mode
                        assert 1 <= w.wait_value <= n_upd
                        waits.append(w)
    keep_idx = sorted({w.wait_value - 1 for w in waits} | {n_upd - 1})
    for w in waits:
        w.wait_value = bisect.bisect_left(keep_idx, w.wait_value - 1) + 1
    keep_set = set(keep_idx)
    n_stripped = 0
    for i, inst in enumerate(updaters):
        if i in keep_set:
            continue
        si = inst.sync_info
        new_upd = [u for u in si.on_update if u.id != sem_id]
        inst.sync_info = mybir.SyncInfo(
            on_wait=list(si.on_wait or []), on_update=new_upd
        )
        n_stripped += 1
    return n_stripped


def get_nc():
    if "nc" not in _NC_CACHE:
        _ensure_paths()
        _NC_CACHE["nc"] = _build_nc()
    return _NC_CACHE["nc"]


def _pmajor(a):
    # [1024, N] -> [128, 8*N] where out[p, k*N+j] = a[k*128+p, j]
    return np.ascontiguousarray(
        a.reshape(KE, P, a.shape[1]).transpose(1, 0, 2).reshape(P, KE * a.shape[1])
    )


def make_in_maps(inputs):
    import ml_dtypes

    bf16 = ml_dtypes.bfloat16
    x = np.asarray(inputs["input"], dtype=np.float32).astype(bf16)
    h = np.asarray(inputs["hidden_state"], dtype=np.float32).astype(bf16)
    xT = x.T  # [E, B]
    hT = h.T
    shared = {
        n: _pmajor(np.asarray(inputs[n], dtype=np.float32).astype(bf16))
        for n in W_NAMES
    }
    shared.update(
        {n: np.ascontiguousarray(np.asarray(inputs[n], dtype=np.float32)) for n in B_NAMES}
    )
    in_maps = []
    for c in range(NCORES):
        sl = slice(c * B_SH, (c + 1) * B_SH)
        m = {
            "inputT": _pmajor(np.ascontiguousarray(xT[:, sl])),
            "hiddenT": _pmajor(np.ascontiguousarray(hT[:, sl])),
            "hiddenN": _pmajor(np.ascontiguousarray(h[sl])),
        }
        m.update(shared)
        in_maps.append(m)
    return in_maps


def kernel(**inputs):
    _ensure_paths()
    from concourse.bass_utils import run_bass_kernel_spmd

    nc = get_nc()
    res = run_bass_kernel_spmd(nc, make_in_maps(inputs), list(range(NCORES)))
    out = np.concatenate(
        [np.asarray(res.results[c]["output"]) for c in range(NCORES)], axis=0
    )
    return out.astype(np.float32)


# revision 27
# speedup vs baseline: 1.0424x; 1.0216x over previous
"""GRU cell kernel for Trainium2, data-parallel across 8 NeuronCores.

Per core: batch shard of 1024 rows; weights replicated.
  u  = sigmoid(x @ Wxu + h @ Whu + bu)
  r  = sigmoid(x @ Wxr + h @ Whr + br)
  c' = tanh  (x @ Wxc + (h*r) @ Whc + bc)
  c  = u*c' + (1-u)*h

The PE does 768 512-col bf16 matmuls (213 ns streaming floor each at
2.4 GHz -> 164 us/core); everything else hides behind that stream,
which runs at its 216 ns/MM issue floor:
  - consecutive matmuls accumulate into the SAME psum bank (k-inner):
    switching the psum write target every matmul costs ~45 ns of PE
    micro-idle (measured via a k-outer variant), which is why
    LDWEIGHTS-sharing loop orders lose
  - a post-pass strips the per-matmul semaphore increment (a
    serialized ~22 ns EVT_SEM write, measured: 235->216 ns/MM) from
    every matmul nobody waits on, renumbering the surviving wait
    thresholds; every wait target stays an updater so no wait can
    point past its producer
  - host pre-casts to bf16 and lays every DRAM tensor out "p-major"
    ([128, 8192], 16 KB contiguous rows) so the bulk loads ride few
    big-descriptor DMAs (~0.42 MB/us aggregate vs ~0.26 with 2 KB
    descriptors); the r working set streams in k-progressive chunks
    on both HWDGE queues, x/h at the same k so both queues drain in
    lockstep; the first ~30 us are HBM-delivery-bound (8 cores pull
    replicated weights simultaneously), so the r gate runs as a
    k-major wave over 7 psum banks -- a wide wave absorbs the most
    matmuls into the unavoidable DMA window, and the 8th bank keeps
    the wave-drain handoff off the critical path
  - warm-up matmuls on a first-arriving 32 KB weight micro-chunk
    (K=128 stationary; K=1 does NOT register as PE-busy for the HAM
    clock gate) ramp the clock to 2.4 GHz while the r set lands
  - r gate runs transposed (W stationary) so its bias is per-partition
    and rh^T lands in the layout the c-gate needs as stationary
  - no SBUF slot reuse for weights (all six stay resident): no WAR
    semaphores against the matmul stream
  - u/c gates run per 128-row chunk with rotating uN/qN pools; the
    final c groups taper to 256 wide so the last drain chain is short,
    with the last store split across both DGE queues
  - fp8 was evaluated and rejected: e4m3 on all gates gives 7e-2 rel
    err, e4m3 on just the r weights 2.1e-2, vs the 2e-2 budget (bf16
    sits at 5.9e-3)
"""

import os
import sys

import numpy as np

B = 8192
E = 1024
H = 1024
NCORES = 8
B_SH = B // NCORES  # 1024 rows per core

P = 128
KE = E // P   # 8 contraction chunks per side
NJ = H // P   # 8 output feature chunks
BN = 512      # moving free-dim per matmul / psum tile width
NB = B_SH // BN  # 2
NWARM = 16

W_NAMES = ("Wxu", "Whu", "Wxr", "Whr", "Wxc", "Whc")
B_NAMES = ("bu", "br", "bc")

_NC_CACHE = {}


def _ensure_paths():
    for p in ("/opt/trn_rl_repo", "/root/.axon_site/_ro/trn_rl_repo"):
        if os.path.isdir(p) and p not in sys.path:
            sys.path.insert(0, p)


def _build_nc():
    import concourse.bass as bass
    import concourse.mybir as mybir
    from concourse.tile import TileContext

    f32 = mybir.dt.float32
    f16 = mybir.dt.float16
    bf16 = mybir.dt.bfloat16
    AF = mybir.ActivationFunctionType

    f8 = mybir.dt.float8e4

    nc = bass.Bass()
    # p-major [128, 8192] layouts: column block k holds chunk k (16KB rows)
    xT_d = nc.dram_tensor("inputT", [P, KE * B_SH], bf16, kind="ExternalInput")
    hT_d = nc.dram_tensor("hiddenT", [P, KE * B_SH], bf16, kind="ExternalInput")
    w_d = {n: nc.dram_tensor(n, [P, KE * H], bf16, kind="ExternalInput") for n in W_NAMES}
    b_d = {n: nc.dram_tensor(n, [1, H], f32, kind="ExternalInput") for n in B_NAMES}
    out_d = nc.dram_tensor("output", [H, B_SH], f32, kind="ExternalOutput")  # transposed; host flips

    def csl(k, inner):  # column slice for chunk k of a p-major tile
        return slice(k * B_SH + inner.start, k * B_SH + inner.stop)

    with TileContext(nc) as tc:
        with (
            tc.tile_pool(name="sb", bufs=1) as sb,
            tc.tile_pool(name="psum", bufs=1, space="PSUM") as pp,
        ):
            xT = sb.tile([P, KE * B_SH], bf16, tag="xT", bufs=1)
            hT = sb.tile([P, KE * B_SH], bf16, tag="hT", bufs=1)
            rhT = sb.tile([P, KE * B_SH], bf16, tag="rhT", bufs=1)
            wsb = {
                n: sb.tile([P, KE * H], bf16, tag=f"w_{n}", bufs=1, name=f"w_{n}")
                for n in W_NAMES
            }

            def psn(name):  # 512-wide psum tile (1 bank)
                return pp.tile([P, BN], f32, tag="mm", bufs=8, name=name)

            # ---- head DMAs: r working set, k-progressive, x/h split across
            # the two HWDGE queues so they drain in lockstep.  A 32KB micro
            # chunk of the r weights lands first to feed the warm-up MMs ----
            nc.sync.dma_start(wsb["Wxr"][:, 0:P], w_d["Wxr"][:, 0:P])
            nc.scalar.dma_start(wsb["Whr"][:, 0:P], w_d["Whr"][:, 0:P])
            nc.sync.dma_start(xT[:, 0:1024], xT_d[:, 0:1024])
            nc.scalar.dma_start(hT[:, 0:1024], hT_d[:, 0:1024])
            nc.sync.dma_start(wsb["Wxr"][:, P:1024], w_d["Wxr"][:, P:1024])
            nc.scalar.dma_start(wsb["Whr"][:, P:1024], w_d["Whr"][:, P:1024])
            CH = ((1024, 2048), (2048, 4096), (4096, 6144), (6144, 8192))
            for lo, hi in CH:
                nc.sync.dma_start(xT[:, lo:hi], xT_d[:, lo:hi])
                nc.sync.dma_start(wsb["Wxr"][:, lo:hi], w_d["Wxr"][:, lo:hi])
                nc.scalar.dma_start(hT[:, lo:hi], hT_d[:, lo:hi])
                nc.scalar.dma_start(wsb["Whr"][:, lo:hi], w_d["Whr"][:, lo:hi])

            # small bias/constant loads (needed from ~35us on)
            ones = sb.tile([1, BN], bf16, tag="ones", bufs=1)
            nc.vector.memset(ones[:], 1.0)
            bias_t = {}
            for nm in B_NAMES:
                bt = sb.tile([P, NJ], f32, tag=f"bias_{nm}", name=f"bias_{nm}", bufs=1)
                nc.scalar.dma_start(
                    bt[:], b_d[nm][0:1, :].rearrange("a (j p) -> p (a j)", p=P)
                )
                bias_t[nm] = bt
            br_t = bias_t["br"]

            # remaining weights, in need-by order
            nc.sync.dma_start(wsb["Wxu"][:], w_d["Wxu"][:])
            nc.sync.dma_start(wsb["Wxc"][:], w_d["Wxc"][:])
            nc.scalar.dma_start(wsb["Whu"][:], w_d["Whu"][:])
            nc.scalar.dma_start(wsb["Whc"][:], w_d["Whc"][:])

            # ---- warm-up: K=128, N=128 matmuls on the first-arriving weight
            # micro chunk ramp HAM toward full clock; dummy activations
            # preload the Sigmoid/Tanh tables ----
            warm = psn("warm")
            warm_o = sb.tile([1, 8], f32, tag="warm_o", bufs=2)
            nc.scalar.activation(warm_o[:], ones[0:1, 0:8], AF.Sigmoid)
            nc.scalar.activation(warm_o[:], ones[0:1, 0:8], AF.Tanh)
            for i in range(NWARM):
                nc.tensor.matmul(
                    warm[:, 0:P], wsb["Wxr"][:, 0:P], wsb["Wxr"][:, 0:P],
                    start=True, stop=True,
                )

            # ---- r gate (transposed out): r^T*h^T into rhT ----
            def r_sigmoid(j, n, ps):
                nsl = slice(n * BN, (n + 1) * BN)
                osl = csl(j, nsl)
                nc.scalar.activation(
                    rhT[:, osl], ps[:], AF.Sigmoid, bias=br_t[:, j : j + 1]
                )
                nc.vector.tensor_mul(rhT[:, osl], rhT[:, osl], hT[:, osl])

            # first half: k-major over a 7-tile wave (14 MMs per k-level,
            # x and h at the same k so both queues feed in lockstep).  7
            # tiles + warm = 8 psum banks: the first serial tile reuses the
            # long-free warm bank, giving the first wave drain a full
            # serial-tile of slack before its bank is needed (an 8-tile
            # wave makes that reuse race the ~2us cross-engine semaphore
            # wake latency and intermittently trips a HAM re-throttle)
            wave = [(j, n) for j in (0, 1, 2, 3) for n in range(NB)][:-1]
            wtiles = {jn: psn(f"ps_r{jn[0]}{jn[1]}") for jn in wave}
            for k in range(KE):
                for j, n in wave:
                    jsl = slice(j * P, (j + 1) * P)
                    nsl = slice(n * BN, (n + 1) * BN)
                    ps = wtiles[(j, n)]
                    nc.tensor.matmul(
                        ps[:], wsb["Wxr"][:, csl(k, jsl)], xT[:, csl(k, nsl)],
                        start=(k == 0), stop=False,
                    )
                    nc.tensor.matmul(
                        ps[:], wsb["Whr"][:, csl(k, jsl)], hT[:, csl(k, nsl)],
                        start=False, stop=(k == KE - 1),
                    )
            for jn in wave:
                r_sigmoid(jn[0], jn[1], wtiles[jn])

            # second half: weights resident; 16 consecutive MMs per psum tile
            def r_serial(j, ns=(0, 1)):
                jsl = slice(j * P, (j + 1) * P)
                for n in ns:
                    nsl = slice(n * BN, (n + 1) * BN)
                    ps = psn(f"ps_r{j}{n}")
                    for k in range(KE):
                        nc.tensor.matmul(
                            ps[:], wsb["Wxr"][:, csl(k, jsl)], xT[:, csl(k, nsl)],
                            start=(k == 0), stop=False,
                        )
                    for k in range(KE):
                        nc.tensor.matmul(
                            ps[:], wsb["Whr"][:, csl(k, jsl)], hT[:, csl(k, nsl)],
                            start=False, stop=(k == KE - 1),
                        )
                    r_sigmoid(j, n, ps)

            r_serial(3, ns=(1,))
            r_serial(4)
            r_serial(5)

            for j in range(6, NJ):
                r_serial(j)

            # ---- u gate then c gate, fully transposed (like r): per
            # (j, n) tile the stationary is the weight column chunk and the
            # moving operand is xT/hT/rhT.  The per-partition biases ride
            # the activation for free, q = h - u*h comes straight from the
            # resident hT, and the output stores transposed (host flips).
            for j in range(NJ):
                jsl = slice(j * P, (j + 1) * P)
                uT = {}
                qT = {}
                for n in range(NB):
                    nsl = slice(n * BN, (n + 1) * BN)
                    ps = psn(f"ps_u{j}{n}")
                    for k in range(KE):
                        nc.tensor.matmul(
                            ps[:], wsb["Wxu"][:, csl(k, jsl)], xT[:, csl(k, nsl)],
                            start=(k == 0), stop=False,
                        )
                    for k in range(KE):
                        nc.tensor.matmul(
                            ps[:], wsb["Whu"][:, csl(k, jsl)], hT[:, csl(k, nsl)],
                            start=False, stop=(k == KE - 1),
                        )
                    ut = sb.tile([P, BN], f16, tag="uT", bufs=3, name=f"uT{j}{n}")
                    qt = sb.tile([P, BN], f16, tag="qT", bufs=3, name=f"qT{j}{n}")
                    nc.scalar.activation(
                        ut[:], ps[:], AF.Sigmoid, bias=bias_t["bu"][:, j : j + 1]
                    )
                    # q = h - u*h  (so the blend is c = u*c' + q)
                    nc.vector.tensor_mul(qt[:], ut[:], hT[:, csl(j, nsl)])
                    nc.vector.tensor_sub(qt[:], hT[:, csl(j, nsl)], qt[:])
                    uT[n] = ut
                    qT[n] = qt
                # c gate + blend + store; the last tile tapers to 256-wide
                # psum groups so the final drain chain is short
                last_j = j == NJ - 1
                groups = ((0, 512), (512, 768), (768, 1024)) if last_j else ((0, 512), (512, 1024))
                for gi, (g0, g1) in enumerate(groups):
                    gw = g1 - g0
                    nsl = slice(g0, g1)
                    n = g0 // BN
                    no = g0 - n * BN
                    ps = psn(f"ps_c{j}{gi}")
                    for k in range(KE):
                        nc.tensor.matmul(
                            ps[:, :gw], wsb["Wxc"][:, csl(k, jsl)], xT[:, csl(k, nsl)],
                            start=(k == 0), stop=False,
                        )
                    for k in range(KE):
                        nc.tensor.matmul(
                            ps[:, :gw], wsb["Whc"][:, csl(k, jsl)], rhT[:, csl(k, nsl)],
                            start=False, stop=(k == KE - 1),
                        )
                    cc = sb.tile([P, BN], f32, tag="cc", bufs=3)
                    nc.scalar.activation(
                        cc[:, :gw], ps[:, :gw], AF.Tanh, bias=bias_t["bc"][:, j : j + 1]
                    )
                    nc.vector.tensor_mul(cc[:, :gw], cc[:, :gw], uT[n][:, no : no + gw])
                    nc.vector.tensor_add(cc[:, :gw], cc[:, :gw], qT[n][:, no : no + gw])
                    if last_j and gi == len(groups) - 1:
                        # split the last store across both DGE queues
                        nc.sync.dma_start(out_d[j * P : j * P + P // 2, nsl], cc[0 : P // 2, :gw])
                        nc.scalar.dma_start(out_d[j * P + P // 2 : (j + 1) * P, nsl], cc[P // 2 :, :gw])
                    else:
                        nc.sync.dma_start(out_d[jsl, nsl], cc[:, :gw])

    _split_matmul_waits(nc, mybir)
    if not os.environ.get("KERNEL_NO_DEDUP"):
        _dedup_ldweights(nc, mybir)
    if not os.environ.get("KERNEL_NO_STRIP"):
        _strip_mm_updates(nc, mybir)
    return nc


def _split_matmul_waits(nc, mybir):
    """Walrus codegen allows only one sync-wait on a Matmult (it lowers to an
    LDW+MM pair).  Spill extra waits onto a PE NoOp placed just before."""
    n_fixed = 0
    blocks = list(nc.m.functions[0].blocks)
    origs = [list(b.instructions) for b in blocks]
    spill_nops = {}  # id(inst) -> [nop insts]
    for orig in origs:
        for inst in orig:
            si = inst.sync_info
            if (
                si is not None
                and si.on_wait
                and len(si.on_wait) > 1
            ):
                waits = list(si.on_wait)
                eng = nc.engines[inst.engine]
                nops = []
                for w in waits[:-1]:
                    nop = eng.nop(hint="waitspill").ins
                    nop.sync_info = mybir.SyncInfo(on_wait=[w], on_update=[])
                    nops.append(nop)
                inst.sync_info = mybir.SyncInfo(
                    on_wait=waits[-1:], on_update=list(si.on_update or [])
                )
                spill_nops[id(inst)] = nops
                n_fixed += 1
    for blk, orig in zip(blocks, origs):
        new_list = []
        for inst in orig:
            if id(inst) in spill_nops:
                new_list.extend(spill_nops[id(inst)])
            new_list.append(inst)
        # rebuilding from `orig` also drops any freshly created nops that
        # bass appended to this block's tail
        blk.instructions[:] = new_list
    return n_fixed


def _dedup_ldweights(nc, mybir):
    """Delete InstLdweights that reload the stationary operand already in
    the PE array (same AP as the previous Ldweights).  The duplicate's
    waits move onto the next PE instruction."""
    n_removed = 0
    for blk in nc.m.functions[0].blocks:
        insts = list(blk.instructions)
        keep = []
        last_sig = None
        pending_waits = []
        for inst in insts:
            tn = type(inst).__name__
            eng = str(inst.engine)
            if eng != "EngineType.PE":
                keep.append(inst)
                continue
            if tn == "InstLdweights":
                sig = (
                    repr(inst.ins[0]),
                    getattr(inst, "is_transpose", None),
                    getattr(inst, "perf_mode", None),
                    getattr(inst, "tile_position", None),
                )
                if sig == last_sig:
                    si = inst.sync_info
                    assert not (si and si.on_update), "dup LDW carries update"
                    if si and si.on_wait:
                        pending_waits.extend(si.on_wait)
                    n_removed += 1
                    continue
                last_sig = sig
                keep.append(inst)
            else:
                if tn not in ("InstMatmult", "InstNoOp", "InstRegisterMove"):
                    last_sig = None
                if pending_waits:
                    si = inst.sync_info
                    w = list(si.on_wait) if si and si.on_wait else []
                    seen = {(x.id, x.wait_value) for x in w}
                    for x in pending_waits:
                        if (x.id, x.wait_value) not in seen:
                            w.append(x)
                            seen.add((x.id, x.wait_value))
                    inst.sync_info = mybir.SyncInfo(
                        on_wait=w,
                        on_update=list(si.on_update or []) if si else [],
                    )
                    pending_waits = []
                keep.append(inst)
        blk.instructions[:] = keep
    # spilled multi-waits may have appeared on Matmults again
    _split_matmul_waits(nc, mybir)
    return n_removed


def _strip_mm_updates(nc, mybir):
    """Every PE Matmult increments the PE engine semaphore (a serialized
    ~22ns EVT_SEM write).  Keep the increment only on matmuls some wait
    actually needs, renumbering all wait thresholds on that semaphore."""
    import bisect

    blocks = list(nc.m.functions[0].blocks)
    sem_id = None
    for blk in blocks:
        for inst in blk.instructions:
            if type(inst).__name__ == "InstMatmult":
                si = inst.sync_info
                if si and si.on_update:
                    assert len(si.on_update) == 1
                    u = si.on_update[0]
                    assert u.update_mode == "sem-inc" and u.update_value == 1
                    if sem_id is None:
                        sem_id = u.id
                    else:
                        assert u.id == sem_id
    if sem_id is None:
        return 0
    updaters = []
    for blk in blocks:
        for inst in blk.instructions:
            si = inst.sync_info
            if si and si.on_update:
                for u in si.on_update:
                    if u.id == sem_id:
                        assert u.update_mode == "sem-inc" and u.update_value == 1
                        updaters.append(inst)
    n_upd = len(updaters)
    waits = []
    for blk in blocks:
        for inst in blk.instructions:
            si = inst.sync_info
            if si and si.on_wait:
                for w in si.on_wait:
                    if w.id == sem_id:
                        assert w.wait_mode == "sem-ge-imm", w.wait_mode
                        assert 1 <= w.wait_value <= n_upd
                        waits.append(w)
    keep_idx = sorted({w.wait_value - 1 for w in waits} | {n_upd - 1})
    for w in waits:
        w.wait_value = bisect.bisect_left(keep_idx, w.wait_value - 1) + 1
    keep_set = set(keep_idx)
    n_stripped = 0
    for i, inst in enumerate(updaters):
        if i in keep_set:
            continue
        si = inst.sync_info
        new_upd = [u for u in si.on_update if u.id != sem_id]
        inst.sync_info = mybir.SyncInfo(
            on_wait=list(si.on_wait or []), on_update=new_upd
        )
        n_stripped += 1
    return n_stripped


def get_nc():
    if "nc" not in _NC_CACHE:
        _ensure_paths()
        _NC_CACHE["nc"] = _build_nc()
    return _NC_CACHE["nc"]


def _pmajor(a):
    # [1024, N] -> [128, 8*N] where out[p, k*N+j] = a[k*128+p, j]
    return np.ascontiguousarray(
        a.reshape(KE, P, a.shape[1]).transpose(1, 0, 2).reshape(P, KE * a.shape[1])
    )


def make_in_maps(inputs):
    import ml_dtypes

    bf16 = ml_dtypes.bfloat16
    x = np.asarray(inputs["input"], dtype=np.float32).astype(bf16)
    h = np.asarray(inputs["hidden_state"], dtype=np.float32).astype(bf16)
    xT = x.T  # [E, B]
    hT = h.T
    shared = {
        n: _pmajor(np.asarray(inputs[n], dtype=np.float32).astype(bf16))
        for n in W_NAMES
    }
    shared.update(
        {n: np.ascontiguousarray(np.asarray(inputs[n], dtype=np.float32)) for n in B_NAMES}
    )
    in_maps = []
    for c in range(NCORES):
        sl = slice(c * B_SH, (c + 1) * B_SH)
        m = {
            "inputT": _pmajor(np.ascontiguousarray(xT[:, sl])),
            "hiddenT": _pmajor(np.ascontiguousarray(hT[:, sl])),
        }
        m.update(shared)
        in_maps.append(m)
    return in_maps


def kernel(**inputs):
    _ensure_paths()
    from concourse.bass_utils import run_bass_kernel_spmd

    nc = get_nc()
    res = run_bass_kernel_spmd(nc, make_in_maps(inputs), list(range(NCORES)))
    out = np.concatenate(
        [np.asarray(res.results[c]["output"]).T for c in range(NCORES)], axis=0
    )
    return out.astype(np.float32)
